# revision 1
# baseline (speedup 1.0000x reference)
"""Trainium2 Bass kernel for windowed multi-head attention with relative
position bias (nn_Conv_Attention_11879879544366).

Shapes (hardcoded): x [2,6,12,12,7,7,256]; 288 windows of 294 tokens each,
8 heads x 32 head-dim. Data-parallel over windows: 36 windows per core on 8
NeuronCores.

Per-window dataflow (all matmuls bf16, fp32 PSUM accumulation):
  xT  = xbar-transpose(x_window)                [256, 294] feature-major
  qT/kT = W.T @ xT; bias added by Pool-engine add-copy (per-partition bcast)
  v   = xT.T @ Wv + bv (token-major, PE bias matmul)  [294, 8*(32+1)] (+ones)
  S^T = kT_h.T @ qT_h per head (K=32 row-packed)      -> PSUM [j, (h,i)]
  expS = ACT exp(S^T)  (softmax max-subtraction skipped: scores are O(1))
  attn = expS * exp(bias^T)  (host-precomputed table, DVE bf16 2x)
  out  = attnT.T @ v_aug  (token-major, ones col gives softmax denominator)
  outn = out * recip(denom)  (DVE, per-partition free-axis broadcast)
  yT   = Wout.T @ xbar-transpose(outn)  -> stored feature-major as bf16;
         host transposes back to token-major fp32.

Window pairs share the 38-row j-tail tile (w0 at partitions 0-37, w1 at
64-101). Emission is software-pipelined 3 pairs deep: iteration p issues
xT loads for pair p+1, projections for pair p, score/exp/mul for pair p-1
and attn@v + output projection for pair p-2, with score-psum groups used
as the metronome and qkv/out chunks interleaved between them so the PE
stream never blocks on ACT draining a PSUM bank.
"""

import os
import sys

sys.path.insert(0, "/opt/trn_rl_repo")

import numpy as np
import ml_dtypes

import concourse.bass as bass
import concourse.tile as tile
from concourse import bacc, mybir
from concourse.bass_utils import run_bass_kernel_spmd

BF16 = mybir.dt.bfloat16
F32 = mybir.dt.float32
F8 = mybir.dt.float8e4
DR = mybir.MatmulPerfMode.DoubleRow
AF = mybir.ActivationFunctionType

N_CORES = 8
WIN_PER_CORE = int(os.environ.get("KERNEL_NWIN", "36"))  # dev knob
_POOLMUL_TIS = tuple(
    int(c) for c in os.environ.get("KERNEL_POOLMUL_TIS", "01234") if c.strip())
NTOK = 294          # 6*7*7 tokens per window
NPAD = 304          # padded to multiple of 16 for xbar transpose
D = 256
H = 8
DH = 32
HI = H * NTOK       # 2352: (head, i) flattened free dim
VA = H * (DH + 1)   # 264: v_aug cols (32 v + 1 ones per head)
JT = [(0, 128), (128, 128), (256, 38)]   # j tiles (offset, len)
IT = [(0, 128), (128, 128), (256, 38)]   # i tiles
GROUPS = [(0, 3), (3, 3), (6, 2)]        # score-psum head groups (3,3,2)

_CACHE = {}


def _build_program():
    nc = bacc.Bacc("TRN2", target_bir_lowering=False, debug=False)

    npairs = WIN_PER_CORE // 2

    xw = nc.dram_tensor("xw", [WIN_PER_CORE, NPAD, D], BF16, kind="ExternalInput").ap()
    wq = nc.dram_tensor("wq", [D, D], BF16, kind="ExternalInput").ap()
    wk = nc.dram_tensor("wk", [D, D], BF16, kind="ExternalInput").ap()
    wv = nc.dram_tensor("wv", [D, D], BF16, kind="ExternalInput").ap()
    wo = nc.dram_tensor("wo", [D, D], BF16, kind="ExternalInput").ap()
    bqc = nc.dram_tensor("bqc", [128, 2], BF16, kind="ExternalInput").ap()
    bkc = nc.dram_tensor("bkc", [128, 2], BF16, kind="ExternalInput").ap()
    bv = nc.dram_tensor("bv", [1, D], BF16, kind="ExternalInput").ap()
    expb = nc.dram_tensor("expb", [3, 128, HI], BF16, kind="ExternalInput").ap()
    # output: feature-major per window (2 chunks of 128 features x 294 toks)
    yw = nc.dram_tensor("yw", [WIN_PER_CORE, 2, 128, NTOK], BF16,
                        kind="ExternalOutput").ap()

    with tile.TileContext(nc) as tc:
        with (
            tc.tile_pool(name="res", bufs=1) as res,
            tc.tile_pool(name="xt", bufs=int(os.environ.get("KERNEL_XT","10"))) as p_xt,
            tc.tile_pool(name="qk", bufs=int(os.environ.get("KERNEL_QK","10"))) as p_qk,
            tc.tile_pool(name="vv", bufs=18) as p_v,
            tc.tile_pool(name="es", bufs=int(os.environ.get("KERNEL_ES","4"))) as p_es,
            tc.tile_pool(name="at", bufs=int(os.environ.get("KERNEL_AT","12"))) as p_at,
            tc.tile_pool(name="oo", bufs=int(os.environ.get("KERNEL_OO","6"))) as p_o,
            tc.tile_pool(name="ps", bufs=2, space=bass.MemorySpace.PSUM) as p_ps,
            tc.tile_pool(name="pm", bufs=2, space=bass.MemorySpace.PSUM) as p_pm,
        ):
            # ---- resident tensors ----
            w_s = {}
            for name, dram in (("wq", wq), ("wk", wk), ("wv", wv), ("wo", wo)):
                t = res.tile([128, 2 * D], BF16, tag=name)
                w_s[name] = t

            def emit_weight_loads():
                for name, dram in (("wq", wq), ("wk", wk), ("wv", wv),
                                   ("wo", wo)):
                    t = w_s[name]
                    for kc in range(2):
                        nc.sync.dma_start(t[:, kc * D:(kc + 1) * D],
                                          dram[kc * 128:(kc + 1) * 128, :])
            b_s = {}
            for name, dram in (("bqc", bqc), ("bkc", bkc)):
                t = res.tile([128, 2], BF16, tag=name)
                nc.gpsimd.dma_start(t[:], dram[:])
                b_s[name] = t
            bv_s = res.tile([1, D], BF16, tag="bv")
            nc.gpsimd.dma_start(bv_s[:], bv[:])
            ones_s = res.tile([1, NPAD], BF16, name="ones", tag="ones")
            nc.gpsimd.memset(ones_s[:], 1.0)
            # big resident loads go out via the otherwise-idle Pool/DVE
            # queues so the SP queue is free for the first xT transposes
            expb_s = []
            for t3 in range(3):
                t = res.tile([128, HI], BF16, name=f"expb{t3}", tag=f"expb{t3}")
                nc.gpsimd.dma_start(t[:], expb[t3])
                expb_s.append(t)

            def hrange(t, h, j0, jl):
                """head-h slice of a [128,588] qT/kT tile: rows 32*(h%4),
                cols 294*(h//4)+j0."""
                return t[32 * (h % 4):32 * (h % 4) + 32,
                         NTOK * (h // 4) + j0:NTOK * (h // 4) + j0 + jl]

            # per-pair live state, keyed by pair index
            state = {}

            def emit_loads(p):
                """Stage 0: xbar-transpose x for both windows of pair p."""
                st = state.setdefault(p, {})
                st["xT"] = {}
                for w in (2 * p, 2 * p + 1):
                    xT = [p_xt.tile([128, NPAD], BF16, name="xT", tag="xT")
                          for _ in range(2)]
                    for kc in range(2):
                        nc.sync.dma_start_transpose(
                            xT[kc][:], xw[w][:, kc * 128:(kc + 1) * 128])
                    st["xT"][w] = xT

            def qkv_chunks(p):
                """Stage 1: q/k/v projections for pair p. Yields closures."""
                st = state[p]
                st["qT"], st["kT"] = {}, {}
                st["v"] = {}
                vsh = p_v.tile([128, VA], BF16, name="vaug", tag="vaug")
                for w in (2 * p, 2 * p + 1):
                    st["qT"][w] = p_qk.tile([128, 2 * NTOK], F8, name="qT", tag="qT")
                    st["kT"][w] = p_qk.tile([128, 2 * NTOK], F8, name="kT", tag="kT")
                    v01 = [p_v.tile([128, VA], BF16, name="vaug", tag="vaug")
                           for _ in range(2)]
                    st["v"][w] = [v01[0], v01[1], vsh]

                def qk_chunk(w, name, c):
                    def go():
                        xT = st["xT"][w]
                        wname = "wq" if name == "q" else "wk"
                        bname = "bqc" if name == "q" else "bkc"
                        dst = (st["qT"] if name == "q" else st["kT"])[w]
                        pq = p_pm.tile([128, 512], F32, name="pm", tag="pm")
                        for kc in range(2):
                            nc.tensor.matmul(
                                pq[:, 0:NTOK],
                                w_s[wname][:, kc * D + c * 128:kc * D + c * 128 + 128],
                                xT[kc][:, 0:NTOK],
                                start=(kc == 0), stop=(kc == 1))
                        # bias add folded into the PSUM->SBUF fp8 cast (DVE;
                        # GPSIMD cannot touch PSUM): bias column broadcast
                        # along the free (token) axis.
                        bcol = b_s[bname][:, c:c + 1]
                        bb = bass.AP(bcol.tensor, bcol.offset,
                                     [list(bcol.ap[0]), [0, NTOK]])
                        nc.vector.tensor_add(
                            dst[:, c * NTOK:(c + 1) * NTOK],
                            pq[:, 0:NTOK], bb)
                    return go

                def v_chunk(w, jt):
                    def go():
                        xT = st["xT"][w]
                        j0, jl = JT[jt]
                        r0 = 64 * (w % 2) if jt == 2 else 0
                        pv = p_pm.tile([128, 512], F32, name="pm", tag="pm")
                        for kc in range(2):
                            nc.tensor.matmul(
                                pv[r0:r0 + jl, 0:D],
                                xT[kc][:, j0:j0 + jl],
                                w_s["wv"][:, kc * D:(kc + 1) * D],
                                start=(kc == 0), stop=False)
                        nc.tensor.matmul(
                            pv[r0:r0 + jl, 0:D],
                            ones_s[0:1, 0:jl],
                            bv_s[0:1, :],
                            start=False, stop=True)
                        vt = st["v"][w][jt]
                        dst = vt[r0:r0 + jl, 0:VA].rearrange(
                            "p (h c) -> p h c", c=DH + 1)
                        nc.vector.tensor_copy(
                            dst[:, :, 0:DH],
                            pv[r0:r0 + jl, 0:D].rearrange("p (h c) -> p h c", c=DH))
                        nc.gpsimd.memset(dst[:, :, DH:DH + 1], 1.0)
                    return go

                for w in (2 * p, 2 * p + 1):
                    for c in range(2):
                        yield qk_chunk(w, "q", c)
                        yield qk_chunk(w, "k", c)
                    for jt in range(3):
                        yield v_chunk(w, jt)

            def score_chunks(p):
                """Stage 2: scores+exp (metronome) and bias-mul for pair p.
                5 tiles: 2 full j-tiles per window + 1 shared tail."""
                st = state[p]
                w0, w1 = 2 * p, 2 * p + 1
                # tile spec: list of (window, jslice) matmul sources + expb idx
                tspec = [
                    ([(w0, JT[0])], 0),
                    ([(w0, JT[1])], 1),
                    ([(w1, JT[0])], 0),
                    ([(w1, JT[1])], 1),
                    ([(w0, JT[2]), (w1, JT[2])], 2),
                ]
                st["attn"] = {w0: [None] * 3, w1: [None] * 3}

                def grp_chunk(ti, gi, es, shared):
                    mms, _eb = tspec[ti]
                    g0, nh = GROUPS[gi]

                    def go():
                        ps = p_ps.tile([128, 1536], F32, name="ps", tag="ps")
                        # PSUM start=True zeroes the full bank region but
                        # only on the partitions this matmul writes, so each
                        # window's tail rows need their own start=True.
                        for mi, (w, (j0, jl)) in enumerate(mms):
                            rowbase = 64 * (w % 2) if jl != 128 else 0
                            qT, kT = st["qT"][w], st["kT"][w]
                            for hh in range(nh):
                                h = g0 + hh
                                ksl = hrange(kT, h, j0, jl)
                                qsl = hrange(qT, h, 0, NTOK)
                                if jl == 128:
                                    # DoubleRow with a stride-0 block dim:
                                    # both k-blocks read the same 32
                                    # features, so the matmul yields 2*S at
                                    # half the column time; the exp scale
                                    # absorbs the 0.5. (Partial-dst tail
                                    # tiles fail the DoubleRow ISA check,
                                    # so they use the plain fp8 path.)
                                    k2 = bass.AP(ksl.tensor, ksl.offset,
                                                 [list(ksl.ap[0]), [0, 2],
                                                  list(ksl.ap[-1])])
                                    q2 = bass.AP(qsl.tensor, qsl.offset,
                                                 [list(qsl.ap[0]), [0, 2],
                                                  list(qsl.ap[-1])])
                                    nc.tensor.matmul(
                                        ps[rowbase:rowbase + jl,
                                           512 * hh:512 * hh + NTOK],
                                        k2, q2,
                                        start=True, stop=True, perf_mode=DR,
                                        tile_position=(32 * (h % 4), rowbase))
                                else:
                                    nc.tensor.matmul(
                                        ps[rowbase:rowbase + jl,
                                           512 * hh:512 * hh + NTOK],
                                        ksl, qsl,
                                        start=True, stop=True,
                                        skip_group_check=(len(mms) > 1),
                                        tile_position=(32 * (h % 4), rowbase))
                        src = ps[:, 0:512 * nh].rearrange(
                            "p (g c) -> p g c", c=512)[:, :, 0:NTOK]
                        sc = (0.5 if len(mms) == 1 else 1.0) * DH ** -0.5
                        nc.scalar.activation(
                            es[:, NTOK * g0:NTOK * (g0 + nh)].rearrange(
                                "p (g c) -> p g c", c=NTOK),
                            src, AF.Exp, scale=float(sc))
                    return go

                def mul_chunk(ti, es):
                    mms, ebi = tspec[ti]

                    def go():
                        at = p_at.tile([128, HI], BF16, name="attn", tag="attn")
                        # bias multiplies ride the otherwise-idle Pool engine
                        # (SBUF-only operands); DVE keeps the PSUM drains
                        eng = nc.gpsimd if ti in _POOLMUL_TIS else nc.vector
                        eng.tensor_mul(at[:], es[:], expb_s[ebi][:])
                        for w, (j0, jl) in mms:
                            jc = j0 // 128 if jl == 128 else 2
                            st["attn"][w][jc] = at
                    return go

                for ti in range(5):
                    es = p_es.tile([128, HI], BF16, name="expS", tag="expS")
                    shared = len(tspec[ti][0]) > 1
                    for gi in range(3):
                        yield grp_chunk(ti, gi, es, shared)
                    yield mul_chunk(ti, es)

            def out_chunks(p):
                """Stage 3: attn@v, normalize, transpose, out-proj, store."""
                st = state[p]
                w0, w1 = 2 * p, 2 * p + 1
                st["outn"] = {w0: [], w1: []}
                st["onT"] = {}
                for w in (w0, w1):
                    st["onT"][w] = [p_o.tile([128, 384], BF16, name="onT",
                                             tag="onT") for _ in range(2)]

                def av_chunk(w, it):
                    def go():
                        i0, il = IT[it]
                        vt = st["v"][w]
                        atiles = st["attn"][w]
                        av = p_pm.tile([128, 512], F32, name="pm", tag="pm")
                        for jc, (j0, jl) in enumerate(JT):
                            r0 = 64 * (w % 2) if jc == 2 else 0
                            for h in range(H):
                                nc.tensor.matmul(
                                    av[0:il, 33 * h:33 * h + 33],
                                    atiles[jc][r0:r0 + jl,
                                               NTOK * h + i0:NTOK * h + i0 + il],
                                    vt[jc][r0:r0 + jl, 33 * h:33 * h + 33],
                                    start=(jc == 0 and h == 0),
                                    stop=(jc == 2 and h == H - 1))
                        rec = p_o.tile([128, H], F32, name="rec", tag="rec")
                        av3 = av[0:il, 0:VA].rearrange("p (h c) -> p h c", c=DH + 1)
                        nc.vector.reciprocal(
                            rec[0:il, :].rearrange("p h -> p h ()"),
                            av3[:, :, DH:DH + 1])
                        on = p_o.tile([128, D], BF16, name="outn", tag="outn")
                        in0 = av3[:, :, 0:DH]
                        in1 = bass.AP(rec.tensor, rec[0:il, 0:1].offset,
                                      [list(rec[0:il, 0:H].ap[0]), [1, H], [0, DH]])
                        nc.vector.tensor_mul(
                            on[0:il, :].rearrange("p (h c) -> p h c", c=DH),
                            in0, in1)
                        st["outn"][w].append(on)
                        for c in range(2):
                            nc.sync.dma_start_transpose(
                                st["onT"][w][c][:, it * 128:(it + 1) * 128],
                                on[:, c * 128:(c + 1) * 128])
                    return go

                def oproj_chunk(w, c):
                    def go():
                        onT = st["onT"][w]
                        py = p_pm.tile([128, 512], F32, name="pm", tag="pm")
                        for kc in range(2):
                            nc.tensor.matmul(
                                py[:, 0:NTOK],
                                w_s["wo"][:, kc * D + c * 128:kc * D + c * 128 + 128],
                                onT[kc][:, 0:NTOK],
                                start=(kc == 0), stop=(kc == 1))
                        yt = p_o.tile([128, NTOK], BF16, name="yT", tag="yT")
                        nc.vector.tensor_copy(yt[:], py[:, 0:NTOK])
                        nc.sync.dma_start(yw[w][c], yt[:])
                    return go

                for it in range(3):
                    yield av_chunk(w0, it)
                for it in range(3):
                    yield av_chunk(w1, it)
                for w in (w0, w1):
                    for c in range(2):
                        yield oproj_chunk(w, c)

            def emit_iteration(p):
                """Pipeline: loads(p+1) | qkv(p) | scores(p-1) | out(p-2),
                with score-psum groups as the metronome and qkv/out chunks
                interleaved between them."""
                if 0 <= p + 1 < npairs:
                    emit_loads(p + 1)
                fillers = []
                if 0 <= p < npairs:
                    fillers.extend(qkv_chunks(p))
                if 0 <= p - 2 < npairs:
                    fillers.extend(out_chunks(p - 2))
                metronome = list(score_chunks(p - 1)) if 0 <= p - 1 < npairs \
                    else []
                fi = iter(fillers)
                done = False
                for i, m in enumerate(metronome):
                    m()
                    if done:
                        continue
                    # ~24 fillers per 20 metronome chunks
                    want = 2 if i % 3 == 0 else 1
                    for _ in range(want):
                        try:
                            next(fi)()
                        except StopIteration:
                            done = True
                            break
                for f in fi:
                    f()

            # prefetch pair-0 x loads ahead of the resident weight loads so
            # the first projections start as early as possible
            emit_weight_loads()
            emit_loads(0)
            for p in range(npairs + 2):
                emit_iteration(p)
                # free stale state
                state.pop(p - 2, None)

    nc.compile()
    return nc


def _rel_position_index():
    d, h, w = 6, 7, 7
    coords = np.stack(np.meshgrid(np.arange(d), np.arange(h), np.arange(w),
                                  indexing="ij"))
    cf = coords.reshape(3, -1)
    rel = cf[:, :, None] - cf[:, None, :]
    rel = rel.transpose(1, 2, 0).copy()
    rel[..., 0] += d - 1
    rel[..., 1] += h - 1
    rel[..., 2] += w - 1
    rel[..., 0] *= (2 * h - 1) * (2 * w - 1)
    rel[..., 1] *= (2 * w - 1)
    return rel.sum(-1)


def kernel(x, Wq, bq, Wkv, bkv, Wout, bias_table):
    bf16 = ml_dtypes.bfloat16
    scale = DH ** -0.5
    b, l, gx, gy, w1, w2, d = x.shape
    B = b * gx * gy
    xt = np.ascontiguousarray(
        np.transpose(x, (0, 2, 3, 1, 4, 5, 6)).reshape(B, NTOK, d))
    xp = np.zeros((B, NPAD, d), dtype=bf16)
    xp[:, :NTOK] = xt.astype(bf16)

    relidx = _rel_position_index()
    bias = bias_table[relidx]                       # [i, j, h] fp32
    eb = np.exp(bias.astype(np.float64)).transpose(1, 2, 0)  # [j, h, i]
    eb = np.ascontiguousarray(eb.reshape(NTOK, HI)).astype(bf16)
    expb3 = np.zeros((3, 128, HI), dtype=bf16)
    expb3[0] = eb[0:128]
    expb3[1] = eb[128:256]
    expb3[2, 0:38] = eb[256:294]
    expb3[2, 64:102] = eb[256:294]

    common = {
        "wq": np.ascontiguousarray(Wq.astype(bf16)),
        "wk": np.ascontiguousarray(Wkv[:, :d].astype(bf16)),
        "wv": np.ascontiguousarray(Wkv[:, d:].astype(bf16)),
        "wo": np.ascontiguousarray(Wout.astype(bf16)),
        "bqc": np.ascontiguousarray(bq.astype(bf16).reshape(2, 128).T),
        "bkc": np.ascontiguousarray(bkv[:d].astype(bf16).reshape(2, 128).T),
        "bv": np.ascontiguousarray(bkv[d:].astype(bf16)[None, :]),
        "expb": expb3,
    }
    wpc = WIN_PER_CORE
    in_maps = [dict(common, xw=np.ascontiguousarray(xp[c * wpc:(c + 1) * wpc]))
               for c in range(N_CORES)]

    if "nc" not in _CACHE:
        _CACHE["nc"] = _build_program()
    res = run_bass_kernel_spmd(_CACHE["nc"], in_maps, list(range(N_CORES)),
                               **_CACHE.get("run_kwargs", {}))
    _CACHE["last_results"] = res
    # yw: [wpc, 2, 128, NTOK] bf16 feature-major -> [win, NTOK, D] fp32
    yw = np.concatenate([np.asarray(res.results[c]["yw"]) for c in range(N_CORES)],
                        axis=0)
    y = yw.reshape(N_CORES * wpc, D, NTOK).astype(np.float32)
    y = np.ascontiguousarray(y.transpose(0, 2, 1))  # token-major
    if WIN_PER_CORE != 36:   # dev mode: raw token-major windows
        return y
    out = y.reshape(b, gx, gy, l, w1, w2, d)
    return np.ascontiguousarray(
        np.transpose(out, (0, 3, 1, 2, 4, 5, 6)).astype(np.float32))



# revision 9
# speedup vs baseline: 1.0601x; 1.0601x over previous
"""Trainium2 Bass kernel for windowed multi-head attention with relative
position bias (nn_Conv_Attention_11879879544366).

Shapes (hardcoded): x [2,6,12,12,7,7,256]; 288 windows of 294 tokens each,
8 heads x 32 head-dim. Data-parallel over windows: 36 windows per core on 8
NeuronCores.

Per-window dataflow (all matmuls bf16, fp32 PSUM accumulation):
  xT  = xbar-transpose(x_window)                [256, 294] feature-major
  qT/kT = W.T @ xT; bias added by Pool-engine add-copy (per-partition bcast)
  v   = xT.T @ Wv + bv (token-major, PE bias matmul)  [294, 8*(32+1)] (+ones)
  S^T = kT_h.T @ qT_h per head (K=32 row-packed)      -> PSUM [j, (h,i)]
  expS = ACT exp(S^T)  (softmax max-subtraction skipped: scores are O(1))
  attn = expS * exp(bias^T)  (host-precomputed table, DVE bf16 2x)
  out  = attnT.T @ v_aug  (token-major, ones col gives softmax denominator)
  outn = out * recip(denom)  (DVE, per-partition free-axis broadcast)
  yT   = Wout.T @ xbar-transpose(outn)  -> stored feature-major as bf16;
         host transposes back to token-major fp32.

Window pairs share the 38-row j-tail tile (w0 at partitions 0-37, w1 at
64-101). Emission is software-pipelined 3 pairs deep: iteration p issues
xT loads for pair p+1, projections for pair p, score/exp/mul for pair p-1
and attn@v + output projection for pair p-2, with score-psum groups used
as the metronome and qkv/out chunks interleaved between them so the PE
stream never blocks on ACT draining a PSUM bank.
"""

import os
import sys

sys.path.insert(0, "/opt/trn_rl_repo")

import numpy as np
import ml_dtypes

import concourse.bass as bass
import concourse.tile as tile
from concourse import bacc, mybir
from concourse.bass_utils import run_bass_kernel_spmd

BF16 = mybir.dt.bfloat16
F32 = mybir.dt.float32
F8 = mybir.dt.float8e4
DR = mybir.MatmulPerfMode.DoubleRow
AF = mybir.ActivationFunctionType

N_CORES = 8
WIN_PER_CORE = int(os.environ.get("KERNEL_NWIN", "36"))  # dev knob
_POOLMUL_TIS = tuple(
    int(c) for c in os.environ.get("KERNEL_POOLMUL_TIS", "04") if c.strip())
_SCORE_DR = os.environ.get("KERNEL_SCORE_DR", "0") == "1"
NTOK = 294          # 6*7*7 tokens per window
NPAD = 304          # padded to multiple of 16 for xbar transpose
D = 256
H = 8
DH = 32
HI = H * NTOK       # 2352: (head, i) flattened free dim
VA = H * (DH + 1)   # 264: v_aug cols (32 v + 1 ones per head)
JT = [(0, 128), (128, 128), (256, 38)]   # j tiles (offset, len)
IT = [(0, 128), (128, 128), (256, 38)]   # i tiles
GROUPS = [(0, 3), (3, 3), (6, 2)]        # score-psum head groups (3,3,2)

_CACHE = {}


def _build_program():
    nc = bacc.Bacc("TRN2", target_bir_lowering=False, debug=False)

    npairs = WIN_PER_CORE // 2

    xw = nc.dram_tensor("xw", [WIN_PER_CORE, NPAD, D], BF16, kind="ExternalInput").ap()
    wq = nc.dram_tensor("wq", [D, D], BF16, kind="ExternalInput").ap()
    wk = nc.dram_tensor("wk", [D, D], BF16, kind="ExternalInput").ap()
    wv = nc.dram_tensor("wv", [D, D], BF16, kind="ExternalInput").ap()
    wo = nc.dram_tensor("wo", [D, D], BF16, kind="ExternalInput").ap()
    bqc = nc.dram_tensor("bqc", [128, 2], BF16, kind="ExternalInput").ap()
    bkc = nc.dram_tensor("bkc", [128, 2], BF16, kind="ExternalInput").ap()
    # cv = Wout.T @ bv folded on host: per-feature bias added at the yT drain
    cvc = nc.dram_tensor("cvc", [128, 2], BF16, kind="ExternalInput").ap()
    expb = nc.dram_tensor("expb", [3, 128, HI], BF16, kind="ExternalInput").ap()
    # output: feature-major per window (2 chunks of 128 features x 294 toks)
    yw = nc.dram_tensor("yw", [WIN_PER_CORE, 2, 128, NTOK], BF16,
                        kind="ExternalOutput").ap()

    with tile.TileContext(nc) as tc:
        with (
            tc.tile_pool(name="res", bufs=1) as res,
            tc.tile_pool(name="xt", bufs=int(os.environ.get("KERNEL_XT","10"))) as p_xt,
            tc.tile_pool(name="qk", bufs=int(os.environ.get("KERNEL_QK","10"))) as p_qk,
            tc.tile_pool(name="vv", bufs=18) as p_v,
            tc.tile_pool(name="es", bufs=int(os.environ.get("KERNEL_ES","4"))) as p_es,
            tc.tile_pool(name="at", bufs=int(os.environ.get("KERNEL_AT","12"))) as p_at,
            tc.tile_pool(name="oo", bufs=int(os.environ.get("KERNEL_OO","6"))) as p_o,
            tc.tile_pool(name="ps", bufs=2, space=bass.MemorySpace.PSUM) as p_ps,
            tc.tile_pool(name="pm", bufs=2, space=bass.MemorySpace.PSUM) as p_pm,
        ):
            # ---- resident tensors ----
            w_s = {}
            for name, dram in (("wq", wq), ("wk", wk), ("wv", wv), ("wo", wo)):
                t = res.tile([128, 2 * D], BF16, tag=name)
                w_s[name] = t

            def emit_weight_loads():
                for name, dram in (("wq", wq), ("wk", wk), ("wv", wv),
                                   ("wo", wo)):
                    t = w_s[name]
                    for kc in range(2):
                        nc.sync.dma_start(t[:, kc * D:(kc + 1) * D],
                                          dram[kc * 128:(kc + 1) * 128, :])
            b_s = {}
            for name, dram in (("bqc", bqc), ("bkc", bkc), ("cvc", cvc)):
                t = res.tile([128, 2], BF16, tag=name)
                nc.gpsimd.dma_start(t[:], dram[:])
                b_s[name] = t
            # big resident loads go out via the otherwise-idle Pool/DVE
            # queues so the SP queue is free for the first xT transposes
            expb_s = []
            for t3 in range(3):
                t = res.tile([128, HI], BF16, name=f"expb{t3}", tag=f"expb{t3}")
                nc.gpsimd.dma_start(t[:], expb[t3])
                expb_s.append(t)

            def hrange(t, h, j0, jl):
                """head-h slice of a [128,588] qT/kT tile: rows 32*(h%4),
                cols 294*(h//4)+j0."""
                return t[32 * (h % 4):32 * (h % 4) + 32,
                         NTOK * (h // 4) + j0:NTOK * (h // 4) + j0 + jl]

            # per-pair live state, keyed by pair index
            state = {}

            def emit_loads(p):
                """Stage 0: xbar-transpose x for both windows of pair p."""
                st = state.setdefault(p, {})
                st["xT"] = {}
                for w in (2 * p, 2 * p + 1):
                    xT = [p_xt.tile([128, NPAD], BF16, name="xT", tag="xT")
                          for _ in range(2)]
                    for kc in range(2):
                        nc.sync.dma_start_transpose(
                            xT[kc][:], xw[w][:, kc * 128:(kc + 1) * 128])
                    st["xT"][w] = xT

            def qkv_chunks(p):
                """Stage 1: q/k/v projections for pair p. Yields closures."""
                st = state[p]
                st["qT"], st["kT"] = {}, {}
                st["v"] = {}
                vsh = p_v.tile([128, VA], BF16, name="vaug", tag="vaug")
                for w in (2 * p, 2 * p + 1):
                    st["qT"][w] = p_qk.tile([128, 2 * NTOK], F8, name="qT", tag="qT")
                    st["kT"][w] = p_qk.tile([128, 2 * NTOK], F8, name="kT", tag="kT")
                    v01 = [p_v.tile([128, VA], BF16, name="vaug", tag="vaug")
                           for _ in range(2)]
                    st["v"][w] = [v01[0], v01[1], vsh]

                def qk_chunk(w, name, c):
                    def go():
                        xT = st["xT"][w]
                        wname = "wq" if name == "q" else "wk"
                        bname = "bqc" if name == "q" else "bkc"
                        dst = (st["qT"] if name == "q" else st["kT"])[w]
                        pq = p_pm.tile([128, 512], F32, name="pm", tag="pm")
                        for kc in range(2):
                            nc.tensor.matmul(
                                pq[:, 0:NTOK],
                                w_s[wname][:, kc * D + c * 128:kc * D + c * 128 + 128],
                                xT[kc][:, 0:NTOK],
                                start=(kc == 0), stop=(kc == 1))
                        # bias add folded into the PSUM->SBUF fp8 cast (DVE;
                        # GPSIMD cannot touch PSUM): bias column broadcast
                        # along the free (token) axis.
                        bcol = b_s[bname][:, c:c + 1]
                        bb = bass.AP(bcol.tensor, bcol.offset,
                                     [list(bcol.ap[0]), [0, NTOK]])
                        nc.vector.tensor_add(
                            dst[:, c * NTOK:(c + 1) * NTOK],
                            pq[:, 0:NTOK], bb)
                    return go

                def v_chunk(w, jt):
                    def go():
                        xT = st["xT"][w]
                        j0, jl = JT[jt]
                        r0 = 64 * (w % 2) if jt == 2 else 0
                        pv = p_pm.tile([128, 512], F32, name="pm", tag="pm")
                        for kc in range(2):
                            nc.tensor.matmul(
                                pv[r0:r0 + jl, 0:D],
                                xT[kc][:, j0:j0 + jl],
                                w_s["wv"][:, kc * D:(kc + 1) * D],
                                start=(kc == 0), stop=(kc == 1))
                        vt = st["v"][w][jt]
                        dst = vt[r0:r0 + jl, 0:VA].rearrange(
                            "p (h c) -> p h c", c=DH + 1)
                        nc.vector.tensor_copy(
                            dst[:, :, 0:DH],
                            pv[r0:r0 + jl, 0:D].rearrange("p (h c) -> p h c", c=DH))
                        nc.gpsimd.memset(dst[:, :, DH:DH + 1], 1.0)
                    return go

                for w in (2 * p, 2 * p + 1):
                    for c in range(2):
                        yield qk_chunk(w, "q", c)
                        yield qk_chunk(w, "k", c)
                    for jt in range(3):
                        yield v_chunk(w, jt)

            def score_chunks(p):
                """Stage 2: scores+exp (metronome) and bias-mul for pair p.
                5 tiles: 2 full j-tiles per window + 1 shared tail."""
                st = state[p]
                w0, w1 = 2 * p, 2 * p + 1
                # tile spec: list of (window, jslice) matmul sources + expb idx
                tspec = [
                    ([(w0, JT[0])], 0),
                    ([(w0, JT[1])], 1),
                    ([(w1, JT[0])], 0),
                    ([(w1, JT[1])], 1),
                    ([(w0, JT[2]), (w1, JT[2])], 2),
                ]
                st["attn"] = {w0: [None] * 3, w1: [None] * 3}

                def grp_chunk(ti, gi, es, shared):
                    mms, _eb = tspec[ti]
                    g0, nh = GROUPS[gi]

                    def go():
                        ps = p_ps.tile([128, 1536], F32, name="ps", tag="ps")
                        # PSUM start=True zeroes the full bank region but
                        # only on the partitions this matmul writes, so each
                        # window's tail rows need their own start=True.
                        for mi, (w, (j0, jl)) in enumerate(mms):
                            rowbase = 64 * (w % 2) if jl != 128 else 0
                            qT, kT = st["qT"][w], st["kT"][w]
                            for hh in range(nh):
                                h = g0 + hh
                                ksl = hrange(kT, h, j0, jl)
                                qsl = hrange(qT, h, 0, NTOK)
                                if jl == 128 and _SCORE_DR:
                                    # DoubleRow with a stride-0 block dim:
                                    # both k-blocks read the same 32
                                    # features, so the matmul yields 2*S at
                                    # half the column time; the exp scale
                                    # absorbs the 0.5. (Partial-dst tail
                                    # tiles fail the DoubleRow ISA check,
                                    # so they use the plain fp8 path.)
                                    k2 = bass.AP(ksl.tensor, ksl.offset,
                                                 [list(ksl.ap[0]), [0, 2],
                                                  list(ksl.ap[-1])])
                                    q2 = bass.AP(qsl.tensor, qsl.offset,
                                                 [list(qsl.ap[0]), [0, 2],
                                                  list(qsl.ap[-1])])
                                    nc.tensor.matmul(
                                        ps[rowbase:rowbase + jl,
                                           512 * hh:512 * hh + NTOK],
                                        k2, q2,
                                        start=True, stop=True, perf_mode=DR,
                                        tile_position=(32 * (h % 4), rowbase))
                                else:
                                    nc.tensor.matmul(
                                        ps[rowbase:rowbase + jl,
                                           512 * hh:512 * hh + NTOK],
                                        ksl, qsl,
                                        start=True, stop=True,
                                        skip_group_check=(len(mms) > 1),
                                        tile_position=(32 * (h % 4), rowbase))
                        src = ps[:, 0:512 * nh].rearrange(
                            "p (g c) -> p g c", c=512)[:, :, 0:NTOK]
                        sc = (0.5 if len(mms) == 1 and _SCORE_DR else 1.0) \
                            * DH ** -0.5
                        nc.scalar.activation(
                            es[:, NTOK * g0:NTOK * (g0 + nh)].rearrange(
                                "p (g c) -> p g c", c=NTOK),
                            src, AF.Exp, scale=float(sc))
                    return go

                def mul_chunk(ti, es):
                    mms, ebi = tspec[ti]

                    def go():
                        at = p_at.tile([128, HI], BF16, name="attn", tag="attn")
                        # bias multiplies ride the otherwise-idle Pool engine
                        # (SBUF-only operands); DVE keeps the PSUM drains
                        eng = nc.gpsimd if ti in _POOLMUL_TIS else nc.vector
                        eng.tensor_mul(at[:], es[:], expb_s[ebi][:])
                        for w, (j0, jl) in mms:
                            jc = j0 // 128 if jl == 128 else 2
                            st["attn"][w][jc] = at
                    return go

                for ti in range(5):
                    es = p_es.tile([128, HI], BF16, name="expS", tag="expS")
                    shared = len(tspec[ti][0]) > 1
                    for gi in range(3):
                        yield grp_chunk(ti, gi, es, shared)
                    yield mul_chunk(ti, es)

            def out_chunks(p):
                """Stage 3: attn@v, normalize, transpose, out-proj, store."""
                st = state[p]
                w0, w1 = 2 * p, 2 * p + 1
                st["outn"] = {w0: [], w1: []}
                st["onT"] = {}
                for w in (w0, w1):
                    st["onT"][w] = [p_o.tile([128, 384], BF16, name="onT",
                                             tag="onT") for _ in range(2)]

                def av_chunk(w, it):
                    def go():
                        i0, il = IT[it]
                        vt = st["v"][w]
                        atiles = st["attn"][w]
                        av = p_pm.tile([128, 512], F32, name="pm", tag="pm")
                        for jc, (j0, jl) in enumerate(JT):
                            r0 = 64 * (w % 2) if jc == 2 else 0
                            for h in range(H):
                                nc.tensor.matmul(
                                    av[0:il, 33 * h:33 * h + 33],
                                    atiles[jc][r0:r0 + jl,
                                               NTOK * h + i0:NTOK * h + i0 + il],
                                    vt[jc][r0:r0 + jl, 33 * h:33 * h + 33],
                                    start=(jc == 0 and h == 0),
                                    stop=(jc == 2 and h == H - 1))
                        rec = p_o.tile([128, H], F32, name="rec", tag="rec")
                        av3 = av[0:il, 0:VA].rearrange("p (h c) -> p h c", c=DH + 1)
                        nc.vector.reciprocal(
                            rec[0:il, :].rearrange("p h -> p h ()"),
                            av3[:, :, DH:DH + 1])
                        on = p_o.tile([128, D], BF16, name="outn", tag="outn")
                        in0 = av3[:, :, 0:DH]
                        in1 = bass.AP(rec.tensor, rec[0:il, 0:1].offset,
                                      [list(rec[0:il, 0:H].ap[0]), [1, H], [0, DH]])
                        nc.vector.tensor_mul(
                            on[0:il, :].rearrange("p (h c) -> p h c", c=DH),
                            in0, in1)
                        st["outn"][w].append(on)
                        for c in range(2):
                            nc.sync.dma_start_transpose(
                                st["onT"][w][c][:, it * 128:(it + 1) * 128],
                                on[:, c * 128:(c + 1) * 128])
                    return go

                def oproj_chunk(w, c):
                    def go():
                        onT = st["onT"][w]
                        py = p_pm.tile([128, 512], F32, name="pm", tag="pm")
                        for kc in range(2):
                            nc.tensor.matmul(
                                py[:, 0:NTOK],
                                w_s["wo"][:, kc * D + c * 128:kc * D + c * 128 + 128],
                                onT[kc][:, 0:NTOK],
                                start=(kc == 0), stop=(kc == 1))
                        yt = p_o.tile([128, NTOK], BF16, name="yT", tag="yT")
                        ccol = b_s["cvc"][:, c:c + 1]
                        cb = bass.AP(ccol.tensor, ccol.offset,
                                     [list(ccol.ap[0]), [0, NTOK]])
                        nc.vector.tensor_add(yt[:], py[:, 0:NTOK], cb)
                        nc.sync.dma_start(yw[w][c], yt[:])
                    return go

                for it in range(3):
                    yield av_chunk(w0, it)
                for it in range(3):
                    yield av_chunk(w1, it)
                for w in (w0, w1):
                    for c in range(2):
                        yield oproj_chunk(w, c)

            def emit_iteration(p):
                """Pipeline: loads(p+1) | qkv(p) | scores(p-1) | out(p-2),
                with score-psum groups as the metronome and qkv/out chunks
                interleaved between them."""
                if 0 <= p + 1 < npairs:
                    emit_loads(p + 1)
                fillers = []
                if 0 <= p < npairs:
                    fillers.extend(qkv_chunks(p))
                if 0 <= p - 2 < npairs:
                    fillers.extend(out_chunks(p - 2))
                metronome = list(score_chunks(p - 1)) if 0 <= p - 1 < npairs \
                    else []
                fi = iter(fillers)
                done = False
                for i, m in enumerate(metronome):
                    m()
                    if done:
                        continue
                    # ~24 fillers per 20 metronome chunks
                    want = 2 if i % 3 == 0 else 1
                    for _ in range(want):
                        try:
                            next(fi)()
                        except StopIteration:
                            done = True
                            break
                for f in fi:
                    f()

            # prefetch pair-0 x loads ahead of the resident weight loads so
            # the first projections start as early as possible
            emit_weight_loads()
            emit_loads(0)
            for p in range(npairs + 2):
                emit_iteration(p)
                # free stale state
                state.pop(p - 2, None)

    nc.compile()
    return nc


def _rel_position_index():
    d, h, w = 6, 7, 7
    coords = np.stack(np.meshgrid(np.arange(d), np.arange(h), np.arange(w),
                                  indexing="ij"))
    cf = coords.reshape(3, -1)
    rel = cf[:, :, None] - cf[:, None, :]
    rel = rel.transpose(1, 2, 0).copy()
    rel[..., 0] += d - 1
    rel[..., 1] += h - 1
    rel[..., 2] += w - 1
    rel[..., 0] *= (2 * h - 1) * (2 * w - 1)
    rel[..., 1] *= (2 * w - 1)
    return rel.sum(-1)


def kernel(x, Wq, bq, Wkv, bkv, Wout, bias_table):
    bf16 = ml_dtypes.bfloat16
    scale = DH ** -0.5
    b, l, gx, gy, w1, w2, d = x.shape
    B = b * gx * gy
    xt = np.ascontiguousarray(
        np.transpose(x, (0, 2, 3, 1, 4, 5, 6)).reshape(B, NTOK, d))
    xp = np.zeros((B, NPAD, d), dtype=bf16)
    xp[:, :NTOK] = xt.astype(bf16)

    relidx = _rel_position_index()
    bias = bias_table[relidx]                       # [i, j, h] fp32
    eb = np.exp(bias.astype(np.float64)).transpose(1, 2, 0)  # [j, h, i]
    eb = np.ascontiguousarray(eb.reshape(NTOK, HI)).astype(bf16)
    expb3 = np.zeros((3, 128, HI), dtype=bf16)
    expb3[0] = eb[0:128]
    expb3[1] = eb[128:256]
    expb3[2, 0:38] = eb[256:294]
    expb3[2, 64:102] = eb[256:294]

    common = {
        "wq": np.ascontiguousarray(Wq.astype(bf16)),
        "wk": np.ascontiguousarray(Wkv[:, :d].astype(bf16)),
        "wv": np.ascontiguousarray(Wkv[:, d:].astype(bf16)),
        "wo": np.ascontiguousarray(Wout.astype(bf16)),
        "bqc": np.ascontiguousarray(bq.astype(bf16).reshape(2, 128).T),
        "bkc": np.ascontiguousarray(bkv[:d].astype(bf16).reshape(2, 128).T),
        "cvc": np.ascontiguousarray(
            (bkv[d:].astype(np.float64) @ Wout.astype(np.float64))
            .astype(bf16).reshape(2, 128).T),
        "expb": expb3,
    }
    wpc = WIN_PER_CORE
    in_maps = [dict(common, xw=np.ascontiguousarray(xp[c * wpc:(c + 1) * wpc]))
               for c in range(N_CORES)]

    if "nc" not in _CACHE:
        _CACHE["nc"] = _build_program()
    res = run_bass_kernel_spmd(_CACHE["nc"], in_maps, list(range(N_CORES)),
                               **_CACHE.get("run_kwargs", {}))
    _CACHE["last_results"] = res
    # yw: [wpc, 2, 128, NTOK] bf16 feature-major -> [win, NTOK, D] fp32
    yw = np.concatenate([np.asarray(res.results[c]["yw"]) for c in range(N_CORES)],
                        axis=0)
    y = yw.reshape(N_CORES * wpc, D, NTOK).astype(np.float32)
    y = np.ascontiguousarray(y.transpose(0, 2, 1))  # token-major
    if WIN_PER_CORE != 36:   # dev mode: raw token-major windows
        return y
    out = y.reshape(b, gx, gy, l, w1, w2, d)
    return np.ascontiguousarray(
        np.transpose(out, (0, 3, 1, 2, 4, 5, 6)).astype(np.float32))

